# revision 5
# baseline (speedup 1.0000x reference)
"""DictionaryLearningOMP forward on 8 TRN2 NeuronCores.

Reference computes out = (pinv(D) @ X).T with D = dictionary.T [256,512],
X = z_e [256,65536].  Equivalently out = X.T @ pinv(dictionary), where
pinv(dictionary) is [256,512].

Sharding: data-parallel along the N=65536 column dim -> 8 shards of 8192
columns.  The small [256,512] pinverse is computed once on host (f64),
scaled by OUT_SCALE, cast to f16 and replicated to every core.

Per-core kernel (PE-bound design, ~27.6us matmul floor at f16):
  - x shard [256,8192] f16 loaded in 7 chunks, triggers on Sync HWDGE
    issued first; dict trigger on Scalar HWDGE (runs in parallel with
    Sync's x0 trigger) so both land ~8.5us.
  - out written TRANSPOSED as [512,8192] float8_e3m4 (4-bit mantissa;
    quantization rel-err ~1.33e-2 vs 2e-2 budget).  Host rescales,
    upcasts and transposes back.
  - matmul: lhsT = dict chunk [128d,128k] stationary, rhs = x window
    [128d,<=512n] moving, PSUM [128k,2,512] f32 (2 banks per tile).
  - PSUM->SBUF cast copies alternate Vector/Scalar engines.
  - PE warm-up (2 matmuls on Vector-memset tiles) rolls seamlessly into
    the real matmuls so the HAM p-state ramp is never reset.
  - tail: last out group is two 256-col windows so the final
    copy->trigger->store chain is short.
"""

import numpy as np

import concourse.bacc as bacc
import concourse.bass as bass
import concourse.mybir as mybir
import concourse.tile as tile
from concourse.bass_utils import run_bass_kernel_spmd

DIM = 256  # contraction dim (data dimension)
KATOMS = 512  # codebook size (output rows in transposed layout)
NTOT = 65536  # total signal columns
NCORES = 8
NSHARD = NTOT // NCORES  # 8192 columns per core

OUT_SCALE = 32.0  # folded into dict on host; out e3m4 holds out*32 (|v|<8.4)

# x load chunks (cols): trigger->data-ready latency is ~3us, so chunks are
# spread over both HWDGE queues (even idx -> Sync, odd idx -> Scalar) to
# stay ahead of a full-speed PE (593 cols/us)
X_CHUNKS = [(0, 512), (512, 512), (1024, 1024), (2048, 2048),
            (4096, 2048), (6144, 2048)]
# (group_start, group_width, [window widths])  -- windows <= 512 (psum bank)
# and aligned so each window lies inside one x chunk
O_GROUPS = [
    (0, 2048, [512, 512, 512, 512]),
    (2048, 2048, [512, 512, 512, 512]),
    (4096, 2048, [512, 512, 512, 512]),
    (6144, 1024, [512, 512]),
    (7168, 512, [512]),
    (7680, 256, [256]),  # short tail: small final copies + tiny stores
    (7936, 256, [256]),
]
NWU = 7  # PE warm-up matmuls: bridge ~7.9us (Tensor free) to ~10.9us (x0
# +dict data ready) seamlessly so the HAM p-state ramp completes in warm-up

LAST_RESULT = None  # BassKernelResults of the most recent run (for test.py)

_cache = {}


def _build_module():
    f32 = mybir.dt.float32
    in_dt = mybir.dt.float16
    out_dt = mybir.dt.float8e3  # e3m4

    nc = bacc.Bacc("TRN2", target_bir_lowering=False, debug=False)

    x = nc.dram_tensor("x0", [DIM, NSHARD], in_dt, kind="ExternalInput")
    dp = nc.dram_tensor("dpt0", [DIM, KATOMS], in_dt, kind="ExternalInput")
    out = nc.dram_tensor("out", [KATOMS, NSHARD], out_dt, kind="ExternalOutput")

    # fold the two 128-row contraction chunks into the partition dim
    xv = x.rearrange("(j p) n -> p j n", p=128)
    # transposed out: partition p holds dict-atom row c*128+p, cols contiguous
    out_v = out.rearrange("(c p) n -> p c n", p=128)

    def chunk_of(n0):
        for ci, (c0, w) in enumerate(X_CHUNKS):
            if c0 <= n0 < c0 + w:
                return ci, n0 - c0
        raise AssertionError(n0)

    with tile.TileContext(nc) as tc:
        with (
            tc.tile_pool(name="dict", bufs=1) as dict_pool,
            tc.tile_pool(name="xin", bufs=1) as xin_pool,
            tc.tile_pool(name="outs", bufs=1) as out_pool,
            tc.tile_pool(name="psum", bufs=3, space=bass.MemorySpace.PSUM) as psum_pool,
            tc.tile_pool(name="wups", bufs=1, space=bass.MemorySpace.PSUM) as wu_pool,
        ):
            # --- load triggers first: x chunks on Sync HWDGE, dict on
            # Scalar HWDGE (parallel trigger paths -> both land ~8.5us) ---
            dpt_sb = dict_pool.tile([128, 2, KATOMS], in_dt, tag="dict")
            nc.scalar.dma_start(dpt_sb[:], dp.rearrange("(j p) k -> p j k", p=128))
            xts = []
            for ci, (c0, w) in enumerate(X_CHUNKS):
                xt = xin_pool.tile([128, 2, w], in_dt, tag=f"x{ci}")
                eng = nc.sync if (ci % 2 == 0) else nc.scalar
                eng.dma_start(xt[:], xv[:, :, c0 : c0 + w])
                xts.append(xt)

            # --- PE warm-up: memset tiles (Vector), NWU dummy matmuls so the
            # HAM p-state ramp starts while the first loads are in flight ---
            wu_lhs = dict_pool.tile([128, 128], in_dt, tag="wu_lhs")
            wu_rhs = dict_pool.tile([128, KATOMS], in_dt, tag="wu_rhs")
            nc.vector.memset(wu_lhs[:], 1.0)
            nc.vector.memset(wu_rhs[:], 1.0)
            wu_ps = wu_pool.tile([128, KATOMS], f32, tag="wu_ps")
            for w in range(NWU):
                nc.tensor.matmul(
                    wu_ps[:], wu_lhs[:], wu_rhs[:],
                    start=(w == 0), stop=(w == NWU - 1),
                )

            # --- main loop ---
            cp_i = 0
            for gi, (g0, gw, wins) in enumerate(O_GROUPS):
                ot = out_pool.tile([128, 4, gw], out_dt, tag=f"o{gi}")
                wo = 0
                for wsz in wins:
                    ci, loc = chunk_of(g0 + wo)
                    xt = xts[ci]
                    for pi in range(2):  # k-chunk pairs (0,1) and (2,3)
                        ps = psum_pool.tile([128, 2, 512], f32)
                        for c2 in range(2):
                            c = pi * 2 + c2
                            for d in range(2):
                                nc.tensor.matmul(
                                    ps[:, c2, :wsz],
                                    dpt_sb[:, d, c * 128 : (c + 1) * 128],
                                    xt[:, d, loc : loc + wsz],
                                    start=(d == 0),
                                    stop=(d == 1),
                                )
                        dst = ot[:, pi * 2 : pi * 2 + 2, wo : wo + wsz]
                        if (cp_i % 2) == 0:
                            nc.vector.tensor_copy(dst, ps[:, :, :wsz])
                        else:
                            nc.scalar.copy(dst, ps[:, :, :wsz])
                        cp_i += 1
                    wo += wsz
                nc.sync.dma_start(out_v[:, :, g0 : g0 + gw], ot[:])

    nc.compile()
    return nc


def _get_module():
    if "m" not in _cache:
        _cache["m"] = _build_module()
    return _cache["m"]


def kernel(z_e, dictionary):
    z_e = np.asarray(z_e, dtype=np.float32)
    dictionary = np.asarray(dictionary, dtype=np.float32)
    assert z_e.shape == (DIM, NTOT), z_e.shape
    assert dictionary.shape == (KATOMS, DIM), dictionary.shape

    # pinv(D).T = pinv(D.T) = pinv(dictionary): [256, 512].  Tiny; computed
    # in f64 on host once, scaled and replicated to all cores.
    dpt = np.linalg.pinv(dictionary.astype(np.float64)) * OUT_SCALE

    nc = _get_module()

    xf16 = z_e.astype(np.float16)
    dpf16 = np.ascontiguousarray(dpt.astype(np.float16))

    in_maps = []
    for i in range(NCORES):
        in_maps.append({
            "x0": np.ascontiguousarray(xf16[:, i * NSHARD : (i + 1) * NSHARD]),
            "dpt0": dpf16,
        })

    res = run_bass_kernel_spmd(nc, in_maps, core_ids=list(range(NCORES)))
    global LAST_RESULT
    LAST_RESULT = res
    outs = [r["out"].astype(np.float32) for r in res.results]  # [512, 8192] each
    full = np.concatenate(outs, axis=1) * (1.0 / OUT_SCALE)  # [512, 65536]
    return np.ascontiguousarray(full.T)
